# revision 28
# baseline (speedup 1.0000x reference)
"""Trainium2 Bass kernel for a batch-of-trees BinaryTreeLSTM.

Contract: kernel(**inputs) takes the FULL inputs (B=128 trees, 1023-node
complete binary tree, dim 300) and returns the FULL output (root_c, root_h),
each [128, 300] float32.

Strategy
--------
- Data-parallel over trees: 16 trees per NeuronCore x 8 cores, no collectives.
- The node scan is really 10 levels (512 leaves -> 256 -> ... -> 1 root).
  Internal nodes have zero input (leaf_mask), so their input-gate term is just
  the bias; the big `inputs` tensor only matters for the 512 leaf rows.
- Everything is computed feature-on-partitions (transposed): states are
  [300, cols] with col = node*16 + tree.  Host pre-transposes the leaf inputs.
- All GEMM operands (x, weights, H state) and the C state are bf16: enables
  fast weight load (FWL), halves DMA/SBUF traffic, psum accumulation is fp32.
- Per level: one GEMM [cols, 600] @ [600, 1500] (i,o,u,fL,fR gates fused,
  weights stationary as lhsT), ACT applies sigmoid/tanh from PSUM (bias baked
  into the GEMM via a ones-row on the rhs), DVE does the elementwise
  recurrence.
- States are stored DEINTERLEAVED ([even nodes | odd nodes]) so the next
  level's children gather is a dense slice; the two 44-partition tail chunks
  of the contraction (features 256:300 of left/right child h) are packed into
  one K-chunk tile at partitions 0:44 / 64:108, giving 5 K-chunks instead of 6.
- ALL intermediate level state is SBUF-resident (bf16 makes it fit); the only
  HBM traffic is the leaf inputs, weights, and the final root state.
"""

import os
import sys

for _p in ("/opt/trn_rl_repo",):
    if os.path.isdir(_p) and _p not in sys.path:
        sys.path.insert(0, _p)

import numpy as np
import ml_dtypes
from contextlib import ExitStack

import concourse.bass as bass
import concourse.tile as tile
from concourse import bacc, mybir
from concourse.bass_utils import run_bass_kernel_spmd

# ---------------------------------------------------------------- constants
N_CORES = 8
B = 128
B_LOC = B // N_CORES          # 16 trees per core
N_LEAVES = 512
MEM = 300
XCOLS = N_LEAVES * B_LOC      # 8192 leaf columns per core
FCH = [(0, 128), (128, 256), (256, 300)]   # feature chunks
NCH = 3
R_LVL = [4096, 2048, 1024, 512, 256, 128, 64, 32, 16]  # cols at levels 1..9
LB = 1024                     # leaf-block columns (64 leaves)
PB_MAX = 512                  # parent-block columns (recurrent levels)

DT = mybir.dt.float32
DTB = mybir.dt.bfloat16
AF = mybir.ActivationFunctionType
GATE_FUNC = [AF.Sigmoid, AF.Sigmoid, AF.Tanh, AF.Sigmoid, AF.Sigmoid]  # i,o,u,fL,fR
BF16 = ml_dtypes.bfloat16


# ---------------------------------------------------------------- host packing
def _pack_weights(Wfioux, b_fioux, Wiouh, Wfh):
    f4 = np.float32
    Wiou = np.asarray(Wfioux[:, 300:1200], f4)            # [300, 900]
    # leaf output columns: six full 128-chunks (gate g in {i,o,u}, half j) at
    # 256*g+128*j, then A = [i2@0|o2@64] at 768:896, B = [u2@0] at 896:940
    Wlp = np.zeros((300, 940), f4)
    for g in range(3):
        for j in range(2):
            Wlp[:, 256 * g + 128 * j: 256 * g + 128 * (j + 1)] =                 Wiou[:, 300 * g + 128 * j: 300 * g + 128 * (j + 1)]
    Wlp[:, 768:812] = Wiou[:, 256:300]    # i2
    Wlp[:, 832:876] = Wiou[:, 556:600]    # o2
    Wlp[:, 896:940] = Wiou[:, 856:900]    # u2
    wleaf = np.zeros((NCH, 128, 940), f4)
    for j, (a, b) in enumerate(FCH):
        wleaf[j, : b - a] = Wlp[a:b]
    Wcat = np.concatenate(
        [Wiouh[:, 0:300], Wiouh[:, 300:600], Wiouh[:, 600:900],
         Wfh[:, 0:300], Wfh[:, 300:600]], axis=1).astype(f4)  # [600, 1500]
    bf = b_fioux[0:300]
    bias_cat = np.concatenate(
        [b_fioux[300:600], b_fioux[600:900], b_fioux[900:1200], bf, bf]).astype(f4)
    # Output-column order for the recurrent GEMM: the ten full 128-wide chunks
    # (gate g, feature half j) at cols 256*g+128*j, then the five 44-wide
    # feature-tail chunks packed 64-aligned so every DVE multiply pair shares
    # a base partition (i2*u2 @0, fL2*CL2 @64, fR2*CR2 @64 with the c2 state
    # stored at partition base 64): chunk A = [i2@0|fL2@64] at 1280:1408,
    # B = [u2@0|fR2@64] at 1408:1536, C = [o2@0] at 1536:1580.
    Wp = np.zeros((600, 1580), f4)
    bp = np.zeros(1580, f4)
    for g in range(5):
        for j in range(2):
            Wp[:, 256 * g + 128 * j: 256 * g + 128 * (j + 1)] = \
                Wcat[:, 300 * g + 128 * j: 300 * g + 128 * (j + 1)]
            bp[256 * g + 128 * j: 256 * g + 128 * (j + 1)] = \
                bias_cat[300 * g + 128 * j: 300 * g + 128 * (j + 1)]
    for dst, sc in ((1280, 256), (1344, 1156), (1408, 856), (1472, 1456),
                    (1536, 556)):
        Wp[:, dst:dst + 44] = Wcat[:, sc:sc + 44]
        bp[dst:dst + 44] = bias_cat[sc:sc + 44]
    wrec = np.zeros((5, 128, 1580), f4)
    wrec[0, :128] = Wp[0:128]
    wrec[1, :128] = Wp[128:256]
    wrec[2, 0:44] = Wp[256:300]          # left-child feat 256:300
    wrec[2, 64:108] = Wp[556:600]        # right-child feat 256:300
    wrec[3, :128] = Wp[300:428]
    wrec[4, :128] = Wp[428:556]
    # bias baked into the recurrent GEMM: ones-row at partition 44 of the
    # packed chunk-2 rhs multiplies this weight row
    wrec[2, 44] = bp
    bias128 = np.zeros((17, 128), f4)
    for g in range(5):
        for j, (a, b) in enumerate(FCH):
            bias128[g * 3 + j, : b - a] = bias_cat[g * 300 + a: g * 300 + b]
    # packed-pair bias columns for the col-tiled 44-wide chunks:
    # col 15: i2 at parts 0:44, o2 at parts 64:108; col 16: fL2 / fR2
    bias128[15, 0:44] = bias_cat[256:300]
    bias128[15, 64:108] = bias_cat[556:600]
    bias128[16, 0:44] = bias_cat[1156:1200]
    bias128[16, 64:108] = bias_cat[1456:1500]
    biasT = np.ascontiguousarray(bias128.T)               # [128, 17]
    return wleaf.astype(BF16), wrec.astype(BF16), biasT


def _check_topology(left_idx, right_idx, leaf_mask):
    li = np.asarray(left_idx); ri = np.asarray(right_idx)
    prev = np.arange(N_LEAVES); nid = N_LEAVES
    ok = bool((np.asarray(leaf_mask)[:N_LEAVES] == 1).all())
    ok &= bool((np.asarray(leaf_mask)[N_LEAVES:] == 0).all())
    while len(prev) > 1:
        cur = []
        for k in range(0, len(prev), 2):
            ok &= bool(li[nid] == prev[k]) and bool(ri[nid] == prev[k + 1])
            cur.append(nid); nid += 1
        prev = np.asarray(cur)
    return ok


def _consts():
    c = np.zeros((130, 2 * LB), np.float32)
    c[44] = 1.0
    return c.astype(BF16)


# ---------------------------------------------------------------- bass program
def _even_odd(ap, half, b=B_LOC):
    """Split a [p, 2*half] block-ordered AP into (even-node cols, odd-node cols),
    each viewed as [p, half//b, b]."""
    r = ap.rearrange("p (m two b) -> p m two b", two=2, b=b)
    return r[:, :, 0, :], r[:, :, 1, :]


def build_program():
    """Builds the full Bass/Tile program. Returns the compiled Bacc."""
    nc = bacc.Bacc("TRN2", target_bir_lowering=False, debug=False)

    xt_d = nc.dram_tensor("xt", [MEM, XCOLS], DTB, kind="ExternalInput").ap()
    wleaf_d = nc.dram_tensor("wleaf", [NCH, 128, 940], DTB, kind="ExternalInput").ap()
    wrec_d = nc.dram_tensor("wrec", [5, 128, 1580], DTB, kind="ExternalInput").ap()
    bias_d = nc.dram_tensor("biasT", [128, 17], DT, kind="ExternalInput").ap()
    zeros_d = nc.dram_tensor("consts", [130, 2 * LB], DTB, kind="ExternalInput").ap()
    out_d = nc.dram_tensor("out", [2, NCH, 128, B_LOC], DT, kind="ExternalOutput").ap()

    with ExitStack() as ctx:
        tc = ctx.enter_context(tile.TileContext(nc))
        _build_kernel(ctx, tc, xt_d, wleaf_d, wrec_d, bias_d, zeros_d, out_d)

    nc.compile()
    return nc


def _build_kernel(ctx, tc, xt_d, wleaf_d, wrec_d, bias_d, zeros_d, out_d):
    nc = tc.nc

    wpool = ctx.enter_context(tc.tile_pool(name="wpool", bufs=1))
    state_pool = ctx.enter_context(tc.tile_pool(name="state", bufs=1))
    tmp_pool = ctx.enter_context(tc.tile_pool(name="tmps", bufs=2))
    out_pool = ctx.enter_context(tc.tile_pool(name="outs", bufs=1))
    # psum pools shared by both phases: 3x 2-bank tiles + 2x 1-bank tiles
    ps2_pool = ctx.enter_context(tc.tile_pool(name="ps2", bufs=3, space="PSUM"))
    ps1_pool = ctx.enter_context(tc.tile_pool(name="ps1", bufs=2, space="PSUM"))

    # ---- weights / bias resident in SBUF
    # bias/wleaf/x-block-0 DMAs are issued BEFORE wrec so the block-0 leaf
    # GEMM can start as early as possible; wrec streams in under it.
    bias_sb = wpool.tile([128, 17], DT, name="bias")
    nc.sync.dma_start(bias_sb[:], bias_d[:])
    # warm-up work that runs during the initial DMA wait: a dummy ACT pulls
    # the sigmoid/tanh table load (~1.3us) off the first real ACT, and a
    # burst of dummy matmuls keeps the PE HAM clock-gate warm so the first
    # real matmuls run at 2.4GHz
    actwarm = wpool.tile([128, 1], DT, name="actwarm")
    nc.scalar.activation(actwarm[:], bias_sb[:, 0:1], AF.Sigmoid)
    hamwarm = wpool.tile([128, 64], DTB, name="hamwarm")
    nc.vector.memset(hamwarm[:], 0)
    ps_w = ps1_pool.tile([128, PB_MAX], mybir.dt.float32, tag="ps1",
                         name="ps_warm")
    for _ in range(48):
        nc.tensor.matmul(ps_w[0:64, 0:64], hamwarm[:, :], hamwarm[:, :],
                         start=True, stop=True)
    wleaf_sb = []
    for k in range(NCH):
        t = wpool.tile([128, 940], DTB, name=f"wleaf{k}")
        nc.sync.dma_start(t[:], wleaf_d[k])
        wleaf_sb.append(t)

    ctx_a = ExitStack()
    xpool = ctx_a.enter_context(tc.tile_pool(name="xpool", bufs=2))
    gate_pool = ctx_a.enter_context(tc.tile_pool(name="gatesA", bufs=2))
    leaf_pool = ctx_a.enter_context(tc.tile_pool(name="leafp", bufs=2))
    gateB_pool_a = ctx_a.enter_context(tc.tile_pool(name="gatesB_a", bufs=2))

    staged_x = {}

    def stage_x(blk):
        c0 = blk * LB
        x_k = []
        for j, (a, b) in enumerate(FCH):
            cw = b - a
            t = xpool.tile([128, LB], DTB, tag=f"x{j}")
            if cw < 128:
                nc.vector.memset(t[32:64, :], 0)
                nc.vector.memset(t[64:128, :], 0)
            nc.sync.dma_start(t[:cw], xt_d[a:b, c0:c0 + LB])
            x_k.append(t)
        staged_x[blk] = x_k

    stage_x(0)
    stage_x(1)

    wrec_sb = []
    for k in range(5):
        t = wpool.tile([128, 1580], DTB, name=f"wrec{k}")
        nc.sync.dma_start(t[:], wrec_d[k])
        wrec_sb.append(t)

    # ---- persistent SBUF state for levels 1..8 (all bf16)
    # H: h0,h1 [128, R] ([left|right]), h2 [128, R//2] (parts 0:44 even-node
    #    feat2, 44 = ones row for the GEMM bias, 64:108 odd-node feat2).
    # C: c0,c1 [128, R], c2 [44, R].
    Hsb = {}; Csb = {}
    zero_fills = []
    for lvl in range(1, 9):
        R = R_LVL[lvl - 1]
        h0 = state_pool.tile([128, R], DTB, name=f"H{lvl}_0")
        h1 = state_pool.tile([128, R], DTB, name=f"H{lvl}_1")
        h2 = state_pool.tile([128, R // 2], DTB, name=f"H{lvl}_2p")
        nc.vector.memset(h2[96:128, :], 0)
        zero_fills.append((h2, R))
        c0 = state_pool.tile([128, R], DTB, name=f"C{lvl}_0")
        c1 = state_pool.tile([128, R], DTB, name=f"C{lvl}_1")
        c2 = state_pool.tile([108, R], DTB, name=f"C{lvl}_2")  # data @64:108
        Hsb[lvl] = (h0, h1, h2)
        Csb[lvl] = (c0, c1, c2)

    def gemm_gates_fused(rhs_k, PBn, gpool, n_free=512):
        """Gate GEMM + ACT: the rhs chunk-2 carries a ones-row at partition 44
        so the GEMM adds the bias; ACTs merge same-function gate pairs.
        j0/j1: psum pair (i|o) [128, 2*PBn], pair (fL|fR), u [128, PBn].
        j2: packed chunks A = [i2|o2|fL2[:40]] and B = [fL2[40:]|fR2|u2].

        Returns (gates, j2_fl, gfull) where gates[g][j] is a [cw, PBn] AP
        (gates[3][2] is None), j2_fl = (fL2[:40] AP, fL2[40:] AP), and
        gfull[j] = the full [128, 2*PBn] (fL|fR) gate AP for j in {0,1}."""
        gates = [[None] * NCH for _ in range(5)]
        gfull = [None, None]

        def mm_cols(ps, w0, w1, base=0):
            cw = w1 - w0
            for n0 in range(0, PBn, n_free):
                n1 = min(n0 + n_free, PBn)
                for kc in range(5):
                    nc.tensor.matmul(
                        ps[base:base + cw, n0:n1],
                        wrec_sb[kc][:, w0:w1],
                        rhs_k[kc][:, n0:n1],
                        start=(kc == 0), stop=(kc == 4))

        def mm_into(ps, g, j, off):
            w0 = 256 * g + 128 * j
            for n0 in range(0, PBn, n_free):
                n1 = min(n0 + n_free, PBn)
                for kc in range(5):
                    nc.tensor.matmul(
                        ps[0:128, off + n0: off + n1],
                        wrec_sb[kc][:, w0:w0 + 128],
                        rhs_k[kc][:, n0:n1],
                        start=(kc == 0), stop=(kc == 4))

        for j in range(2):
            ps_io = ps2_pool.tile([128, 2 * PB_MAX], mybir.dt.float32, tag="ps2",
                                  name="ps_io")
            mm_into(ps_io, 0, j, 0)
            mm_into(ps_io, 1, j, PBn)
            g_io = gpool.tile([128, 2 * PBn], DTB, tag="gate_io", name=f"g_io_{j}")
            nc.scalar.activation(g_io[:], ps_io[:, :2 * PBn], AF.Sigmoid)

            ps_f = ps2_pool.tile([128, 2 * PB_MAX], mybir.dt.float32, tag="ps2",
                                 name="ps_f")
            mm_into(ps_f, 3, j, 0)
            mm_into(ps_f, 4, j, PBn)
            g_f = gpool.tile([128, 2 * PBn], DTB, tag="gate_f", name=f"g_f_{j}")
            nc.scalar.activation(g_f[:], ps_f[:, :2 * PBn], AF.Sigmoid)

            gates[0][j] = g_io[:, :PBn]
            gates[1][j] = g_io[:, PBn:]
            gates[3][j] = g_f[:, :PBn]
            gates[4][j] = g_f[:, PBn:]
            gfull[j] = g_f[:]

        # u gates for both halves share one 2-bank psum tile -> one tanh ACT
        ps_u = ps2_pool.tile([128, 2 * PB_MAX], mybir.dt.float32, tag="ps2",
                             name="ps_u01")
        mm_into(ps_u, 2, 0, 0)
        mm_into(ps_u, 2, 1, PBn)
        g_u = gpool.tile([128, 2 * PBn], DTB, tag="gate_u01", name="g_u01")
        nc.scalar.activation(g_u[:], ps_u[:, :2 * PBn], AF.Tanh)
        gates[2][0] = g_u[:, :PBn]
        gates[2][1] = g_u[:, PBn:]

        # j2: packed 64-aligned chunks A=[i2@0|fL2@64], B=[u2@0|fR2@64], C=[o2]
        ps_a = ps1_pool.tile([128, PB_MAX], mybir.dt.float32, tag="ps1",
                             name="ps_j2a")
        mm_cols(ps_a, 1280, 1408)
        g_a = gpool.tile([128, PBn], DTB, tag="gate_u", name="g_j2a")
        nc.scalar.activation(g_a[0:108, :], ps_a[0:108, :PBn], AF.Sigmoid)
        ps_b = ps1_pool.tile([128, PB_MAX], mybir.dt.float32, tag="ps1",
                             name="ps_j2b")
        mm_cols(ps_b, 1408, 1536)
        g_b = gpool.tile([128, PBn], DTB, tag="gate_b", name="g_j2b")
        nc.scalar.activation(g_b[0:44, :], ps_b[0:44, :PBn], AF.Tanh)
        nc.scalar.activation(g_b[64:108, :], ps_b[64:108, :PBn], AF.Sigmoid)
        ps_c = ps1_pool.tile([128, PB_MAX], mybir.dt.float32, tag="ps1",
                             name="ps_j2c")
        mm_cols(ps_c, 1536, 1580)
        g_c = gpool.tile([44, PBn], DTB, tag="gate_c", name="g_j2c")
        nc.scalar.activation(g_c[:], ps_c[0:44, :PBn], AF.Sigmoid)
        gates[0][2] = g_a[0:44, :]
        gates[1][2] = g_c[:]
        gates[2][2] = g_b[0:44, :]
        gates[3][2] = g_a[64:108, :]
        gates[4][2] = g_b[64:108, :]
        return gates, gfull

    def leaf_gemm_gates(x_k, PBn, n_free=512):
        """Leaf i,o,u gates from x chunks (3 K-chunks), bias via the ACT bias
        operand; the 44-wide feature tails run as packed 64-aligned chunks
        A = [i2@0|o2@64] (one sigmoid ACT, bias col 15) and B = [u2@0]."""
        gates = [[None] * NCH for _ in range(3)]

        def mm_cols(ps, w0, w1):
            cw = w1 - w0
            for n0 in range(0, PBn, n_free):
                n1 = min(n0 + n_free, PBn)
                for kc in range(NCH):
                    nc.tensor.matmul(
                        ps[0:cw, n0:n1],
                        wleaf_sb[kc][:, w0:w1],
                        x_k[kc][:, n0:n1],
                        start=(kc == 0), stop=(kc == NCH - 1))

        for j in range(2):
            for g in range(3):
                ps = ps2_pool.tile([128, 2 * PB_MAX], mybir.dt.float32,
                                   tag="ps2", name="psL")
                mm_cols(ps, 256 * g + 128 * j, 256 * g + 128 * (j + 1))
                gt = gate_pool.tile([128, PBn], DTB, tag=f"lgate{g}", name=f"lgate{g}_{j}")
                m = g * 3 + j
                nc.scalar.activation(gt[:], ps[:128, :PBn], GATE_FUNC[g],
                                     bias=bias_sb[:128, m:m + 1])
                gates[g][j] = gt
        ps_a = ps2_pool.tile([128, 2 * PB_MAX], mybir.dt.float32, tag="ps2",
                             name="psL_a")
        mm_cols(ps_a, 768, 896)
        g_a = gate_pool.tile([128, PBn], DTB, tag="lgate0", name="lg_a")
        nc.scalar.activation(g_a[0:108, :], ps_a[0:108, :PBn], AF.Sigmoid,
                             bias=bias_sb[0:108, 15:16])
        ps_u = ps2_pool.tile([128, 2 * PB_MAX], mybir.dt.float32, tag="ps2",
                             name="psL_u2")
        mm_cols(ps_u, 896, 940)
        g_u = gate_pool.tile([44, PBn], DTB, tag="lgate2", name="lg_u2")
        nc.scalar.activation(g_u[:], ps_u[:44, :PBn], AF.Tanh,
                             bias=bias_sb[:44, 8:9])
        gates[0][2] = g_a[0:44, :]
        gates[1][2] = g_a[64:108, :]
        gates[2][2] = g_u[:]
        return gates

    def write_split(dst_even, dst_odd, in0, in1, op):
        """out = in0 <op> in1, writing even-node cols to dst_even and odd-node
        cols to dst_odd (both dense [cw, PBn//2] APs). in0/in1 are block-dense
        [cw, PBn] APs."""
        cw, PBn = in0.shape[0], in0.shape[1]
        half = PBn // 2
        e0, o0 = _even_odd(in0, half)
        e1, o1 = _even_odd(in1, half)
        de = dst_even.rearrange("p (m b) -> p m b", b=B_LOC)
        do = dst_odd.rearrange("p (m b) -> p m b", b=B_LOC)
        nc.vector.tensor_tensor(de, e0, e1, op)
        nc.vector.tensor_tensor(do, o0, o1, op)

    MUL = mybir.AluOpType.mult
    ADD = mybir.AluOpType.add

    def recur_elementwise(gates, gfull, CL, CR, CLR, PBn, c_dst, h_dst,
                          h2_odd_dst, split=True):
        """Elementwise part for one block of an internal level.

        gates/gfull from gemm_gates_fused.  CL[j]/CR[j]: dense [cw, PBn]
        child-C APs (left/right); CLR[j] (j<2): [cw, 2, PBn] AP covering both.
        c_dst: per-j (even_ap, odd_ap, full_ap) with full_ap an AP covering
        both halves in storage order (may be a 2-window view of a persistent
        tile).  h_dst: per-j (even_ap, odd_ap).
        h2_odd_dst: None, or the AP for the chunk2 odd half (parts 64:108 of
        the packed h2 tile).
        split=False (root level): c_dst/h_dst are (full_ap, None, full_ap); no
        deinterleave is applied."""
        for j in range(NCH):
            cw = FCH[j][1] - FCH[j][0]
            fc = tmp_pool.tile([cw, PBn], DTB, tag="fc", name=f"fc_{j}")
            if j < 2:
                # fused f*c over (left|right) in one op
                tf = tmp_pool.tile([cw, 2 * PBn], DTB, tag="tf", name=f"tf_{j}")
                nc.vector.tensor_tensor(
                    tf[:].rearrange("p (two n) -> p two n", two=2),
                    gfull[j].rearrange("p (two n) -> p two n", two=2),
                    CLR[j], MUL)
                nc.vector.tensor_tensor(fc[:], tf[:, :PBn], tf[:, PBn:], ADD)
            else:
                t1 = tmp_pool.tile([44, PBn], DTB, tag="t1", name="t1_2")
                nc.vector.tensor_tensor(t1[:], gates[3][2][:], CL[2], MUL)
                t2 = tmp_pool.tile([44, PBn], DTB, tag="t2", name="t2_2")
                nc.vector.tensor_tensor(t2[:], gates[4][2][:], CR[2], MUL)
                nc.vector.tensor_tensor(fc[:], t1[:], t2[:], ADD)
            iu = tmp_pool.tile([cw, PBn], DTB, tag="iu", name=f"iu_{j}")
            nc.vector.tensor_tensor(iu[:], gates[0][j][:], gates[2][j][:], MUL)
            ce, co, cfull = c_dst[j]
            he, ho = h_dst[j]
            th = tmp_pool.tile([cw, PBn], DTB, tag="th", name=f"th_{j}")
            if not split:
                nc.vector.tensor_tensor(ce, iu[:], fc[:], ADD)
                nc.scalar.activation(th[:], cfull, AF.Tanh)
                nc.vector.tensor_tensor(he, gates[1][j][:], th[:], MUL)
                continue
            # c (split write into storage order)
            write_split(ce, co, iu[:], fc[:], ADD)
            # tanh(c) over the split-ordered pair; input may be a 2-window view
            half = PBn // 2
            nc.scalar.activation(
                th[:].rearrange("p (two h) -> p two h", two=2), cfull, AF.Tanh)
            # h = o * tanh(c): o is block-dense; th halves are storage-ordered
            e_o, o_o = _even_odd(gates[1][j][:], half)
            nc.vector.tensor_tensor(
                he.rearrange("p (m b) -> p m b", b=B_LOC), e_o,
                th[:, :half].rearrange("p (m b) -> p m b", b=B_LOC), MUL)
            od = h2_odd_dst if (j == 2 and h2_odd_dst is not None) else ho
            nc.vector.tensor_tensor(
                od.rearrange("p (m b) -> p m b", b=B_LOC), o_o,
                th[:, half:].rearrange("p (m b) -> p m b", b=B_LOC), MUL)

    def state_dsts(lvl, p0, PBn):
        """(c_dst, h_dst, h2_odd) triples pointing into level `lvl`'s
        persistent tiles for a block of PBn parent cols starting at p0."""
        R = R_LVL[lvl - 1]
        qh = PBn // 2
        w0 = p0 // 2
        h0, h1, h2 = Hsb[lvl]
        c0, c1, c2 = Csb[lvl]
        c_dst = []
        for j, ct in enumerate((c0, c1, c2)):
            cv = ct[:128] if j < 2 else ct[64:108]
            full = cv.rearrange("p (two h) -> p two h", two=2)[:, :, w0:w0 + qh]
            c_dst.append((cv[:, w0:w0 + qh], cv[:, R // 2 + w0: R // 2 + w0 + qh],
                          full))
        h_dst = [(h0[:128, w0:w0 + qh], h0[:128, R // 2 + w0: R // 2 + w0 + qh]),
                 (h1[:128, w0:w0 + qh], h1[:128, R // 2 + w0: R // 2 + w0 + qh]),
                 (h2[0:44, w0:w0 + qh], None)]
        return c_dst, h_dst, h2[64:108, w0:w0 + qh]

    # ================================================================ phase A
    # leaves + level-1, software-pipelined: the level-1 GEMM for block k-1 is
    # emitted after block k's leaf GEMM so the tensor engine never stalls on
    # the leaf elementwise chain.
    n_lblk = XCOLS // LB                       # 8 blocks

    def l1_block(lh, lc, blk):
        PBn = LB // 2                          # 512 parent cols
        rhs_k = [lh[0][:, :PBn], lh[1][:, :PBn], lh[2][:, :PBn],
                 lh[0][:, PBn:PBn * 2], lh[1][:, PBn:PBn * 2]]
        gates, gfull = gemm_gates_fused(rhs_k, PBn, gateB_pool_a)
        CLs = [lc[0][:128, :PBn], lc[1][:128, :PBn], lc[2][64:108, :PBn]]
        CRs = [lc[0][:128, PBn:], lc[1][:128, PBn:], lc[2][64:108, PBn:]]
        CLRs = [lc[0][:128, :].rearrange("p (two n) -> p two n", two=2),
                lc[1][:128, :].rearrange("p (two n) -> p two n", two=2)]
        c_dst, h_dst, h2_odd = state_dsts(1, blk * PBn, PBn)
        recur_elementwise(gates, gfull, CLs, CRs, CLRs, PBn,
                          c_dst, h_dst, h2_odd)

    pend = None
    for blk in range(n_lblk):
        # --- leaf gates (x was staged in the previous iteration)
        x_k = staged_x.pop(blk)
        lg = leaf_gemm_gates(x_k, LB)
        if blk + 2 < n_lblk:
            stage_x(blk + 2)
        # --- leaf elementwise -> leaf H/C (deinterleaved, block-local)
        half = LB // 2
        lh = [leaf_pool.tile([128, LB], DTB, tag="lh0", name="lh0"),
              leaf_pool.tile([128, LB], DTB, tag="lh1", name="lh1"),
              leaf_pool.tile([128, half], DTB, tag="lh2p", name="lh2p")]
        lc = [leaf_pool.tile([128, LB], DTB, tag="lc0", name="lc0"),
              leaf_pool.tile([128, LB], DTB, tag="lc1", name="lc1"),
              leaf_pool.tile([108, LB], DTB, tag="lc2", name="lc2")]  # @64:108
        nc.sync.dma_start(lh[2][44:64, :], zeros_d[44:64, :half])  # ones row @44
        nc.vector.memset(lh[2][96:128, :], 0)
        for j in range(NCH):
            cw = FCH[j][1] - FCH[j][0]
            lcv = lc[j][:cw] if j < 2 else lc[j][64:108]
            # c = i * u, split write
            write_split(lcv[:, :half], lcv[:, half:],
                        lg[0][j][:], lg[2][j][:], MUL)
            th_t = tmp_pool.tile([64 + cw if j == 2 else cw, LB], DTB,
                                 tag="lth", name=f"lth_{j}")
            th = th_t[64:64 + cw] if j == 2 else th_t[:cw]
            nc.scalar.activation(th[:, :], lcv[:, :], AF.Tanh)
            e_o, o_o = _even_odd(lg[1][j][:], half)
            if j == 2:
                nc.vector.tensor_tensor(
                    lh[2][:cw, :].rearrange("p (m b) -> p m b", b=B_LOC), e_o,
                    th[:, :half].rearrange("p (m b) -> p m b", b=B_LOC), MUL)
                nc.vector.tensor_tensor(
                    lh[2][64:64 + cw, :].rearrange("p (m b) -> p m b", b=B_LOC), o_o,
                    th[:, half:].rearrange("p (m b) -> p m b", b=B_LOC), MUL)
            else:
                nc.vector.tensor_tensor(
                    lh[j][:cw, :half].rearrange("p (m b) -> p m b", b=B_LOC), e_o,
                    th[:, :half].rearrange("p (m b) -> p m b", b=B_LOC), MUL)
                nc.vector.tensor_tensor(
                    lh[j][:cw, half:].rearrange("p (m b) -> p m b", b=B_LOC), o_o,
                    th[:, half:].rearrange("p (m b) -> p m b", b=B_LOC), MUL)

        # --- level-1 for the previous block (pipelined)
        if pend is not None:
            l1_block(*pend)
        pend = (lh, lc, blk)

    l1_block(*pend)
    # ones/zero rows of the persistent h2 tiles (deferred so these DMAs don't
    # sit ahead of the x stages in the queue; levels only need them by the
    # time phase B reads the tile)
    for h2, R in zero_fills:
        nc.sync.dma_start(h2[44:64, :], zeros_d[44:64, : R // 2])
    ctx_a.close()

    # ================================================================ phase B
    # levels 2..9, all SBUF-resident
    ctx_b = ExitStack()
    gateB_pool = ctx_b.enter_context(tc.tile_pool(name="gatesB", bufs=3))
    for lvl in range(2, 10):
        R = R_LVL[lvl - 1]          # this level's column count
        Rp = R_LVL[lvl - 2]         # previous level's column count
        PBn = min(PB_MAX, R)
        for blk in range(R // PBn):
            p0 = blk * PBn
            # ---- children APs from the previous level's persistent tiles
            h0, h1, h2 = Hsb[lvl - 1]
            cc0, cc1, cc2 = Csb[lvl - 1]
            hw = Rp // 2
            rhs_k = [h0[:, p0:p0 + PBn], h1[:, p0:p0 + PBn], h2[:, p0:p0 + PBn],
                     h0[:, hw + p0: hw + p0 + PBn], h1[:, hw + p0: hw + p0 + PBn]]
            CLs = [cc0[:128, p0:p0 + PBn], cc1[:128, p0:p0 + PBn],
                   cc2[64:108, p0:p0 + PBn]]
            CRs = [cc0[:128, hw + p0: hw + p0 + PBn],
                   cc1[:128, hw + p0: hw + p0 + PBn],
                   cc2[64:108, hw + p0: hw + p0 + PBn]]
            CLRs = [
                cc0[:128].rearrange("p (two h) -> p two h", two=2)[:, :, p0:p0 + PBn],
                cc1[:128].rearrange("p (two h) -> p two h", two=2)[:, :, p0:p0 + PBn]]

            gates, gfull = gemm_gates_fused(rhs_k, PBn, gateB_pool)

            if lvl == 9:
                # root: single node -> no deinterleave; all three feature
                # chunks land in one staging tile so c and h each ship in a
                # single DMA (out layout is [2, chunk, 128, tree])
                oc_all = out_pool.tile([128, NCH * PBn], DT, name="oc_all")
                oh_all = out_pool.tile([128, NCH * PBn], DT, name="oh_all")
                nc.vector.memset(oc_all[:, 2 * PBn:], 0)
                nc.vector.memset(oh_all[:, 2 * PBn:], 0)
                def _sl(t, j):
                    cw = FCH[j][1] - FCH[j][0]
                    return t[:cw, j * PBn:(j + 1) * PBn]
                c_dst = [(_sl(oc_all, j), None, _sl(oc_all, j)) for j in range(NCH)]
                h_dst = [(_sl(oh_all, j), None) for j in range(NCH)]
                recur_elementwise(gates, gfull, CLs, CRs, CLRs, PBn,
                                  c_dst, h_dst, None, split=False)
                nc.sync.dma_start(
                    out_d[0].rearrange("j p b -> p j b"),
                    oc_all[:].rearrange("p (j b) -> p j b", b=B_LOC))
                nc.sync.dma_start(
                    out_d[1].rearrange("j p b -> p j b"),
                    oh_all[:].rearrange("p (j b) -> p j b", b=B_LOC))
            else:
                c_dst, h_dst, h2_odd = state_dsts(lvl, p0, PBn)
                recur_elementwise(gates, gfull, CLs, CRs, CLRs, PBn,
                                  c_dst, h_dst, h2_odd)
    ctx_b.close()


# ---------------------------------------------------------------- runner
_CACHE = {}


def _get_program():
    if "nc" not in _CACHE:
        _CACHE["nc"] = build_program()
    return _CACHE["nc"]


def kernel(inputs, Wfioux, b_fioux, Wiouh, Wfh, left_idx, right_idx, leaf_mask,
           _trace=False, _trace_dir=None):
    inputs = np.asarray(inputs, np.float32)
    assert _check_topology(left_idx, right_idx, leaf_mask), \
        "tree topology does not match the expected complete binary tree"

    wleaf, wrec, biasT = _pack_weights(
        np.asarray(Wfioux, np.float32), np.asarray(b_fioux, np.float32),
        np.asarray(Wiouh, np.float32), np.asarray(Wfh, np.float32))

    in_maps = []
    for core in range(N_CORES):
        x = inputs[core * B_LOC:(core + 1) * B_LOC, :N_LEAVES, :]
        xt = np.ascontiguousarray(
            x.transpose(2, 1, 0).reshape(MEM, XCOLS)).astype(BF16)
        in_maps.append({"xt": xt, "wleaf": wleaf, "wrec": wrec, "biasT": biasT,
                        "consts": _consts()})

    nc = _get_program()
    res = run_bass_kernel_spmd(nc, in_maps, list(range(N_CORES)),
                               trace=_trace, tmpdir=_trace_dir)

    root_c = np.zeros((B, MEM), np.float32)
    root_h = np.zeros((B, MEM), np.float32)
    for core in range(N_CORES):
        out = res.results[core]["out"]          # [2, 3, 128, 16]
        sl = slice(core * B_LOC, (core + 1) * B_LOC)
        for dst, t in ((root_c, out[0]), (root_h, out[1])):
            for j, (a, b) in enumerate(FCH):
                dst[sl, a:b] = t[j, : b - a].T
    _CACHE["last_results"] = res
    return root_c, root_h
